# revision 1
# baseline (speedup 1.0000x reference)
"""Self-contained TGCN kernel for the grading harness.

kernel(**inputs) -> np.ndarray [1, 1]

All logic lives in the embedded module below (same code as
work/tgcn_kernel.py at its final state); the harness only needs this file.
"""
import sys

if '/opt/trn_rl_repo' not in sys.path:
    sys.path.insert(0, '/opt/trn_rl_repo')

import numpy as np

# The full implementation is appended below by build tooling.
